# revision 13
# baseline (speedup 1.0000x reference)
"""Gated cosine-affinity kernel for Trainium2 (Bass/Tile), 8-core SPMD.

Problem: for each batch b (B=8):
    Xg = A_1 * X;  Yg = A_2 * Y            (elementwise gates)
    out[b] = normalize_rows(Xg) @ normalize_rows(Yg).T      (2048 x 2048)
with row norm = sqrt(max(|row|^2, 1e-6)).

Sharding: data-parallel over batch — one batch element per NeuronCore.

Per-core structure (memory-bound: ~21 MB HBM traffic vs ~360 GB/s/core):
  stage 1: gate X/Y (DVE+GpSimd), row sum-squares (ACT Square+accum),
           Newton-refined 1/sqrt, PE-transpose into d-major layout.
           X uses a row-permuted contiguous layout (partition p holds rows
           16p..16p+15) so its loads are fully contiguous; the permutation
           is undone for free by a strided store access pattern.
  stage 2: column-slice-major (m-major) matmul order so stores start as
           soon as the first 4 Y chunks are transposed; X's 1/norm is
           folded into the PSUM->SBUF evacuation as a per-partition scale.
           Operands are float32r (1 row/cycle vs 4 for fp32).
"""

import numpy as np
from contextlib import ExitStack

import concourse.bass as bass
import concourse.tile as tile
from concourse import bacc, mybir
from concourse.bass_utils import run_bass_kernel_spmd
from concourse.masks import make_identity

B = 8
N = 2048          # rows of X (output rows)
M = 2048          # rows of Y (output cols)
D = 128           # feature dim == partition count == contraction dim
P = 128
EPS = 1e-6
NCH = N // P      # 16 row-chunks per tensor
NG = 4            # Y chunks per norm-group / per output column-slice
MM_N = 512        # matmul moving free dim (one PSUM bank of fp32)
NMM = M // MM_N   # 4 column-slices
SROW = NCH        # row-permutation stride for X layout

FP32 = mybir.dt.float32
FP32R = mybir.dt.float32r
AF = mybir.ActivationFunctionType

_CACHED_NC = None


def _build_program():
    nc = bacc.Bacc("TRN2", target_bir_lowering=False, debug=False, num_devices=B)

    Xd = nc.dram_tensor("X", [N, D], FP32, kind="ExternalInput")
    Yd = nc.dram_tensor("Y", [M, D], FP32, kind="ExternalInput")
    A1d = nc.dram_tensor("A_1", [N, D], FP32, kind="ExternalInput")
    A2d = nc.dram_tensor("A_2", [M, D], FP32, kind="ExternalInput")
    OUT = nc.dram_tensor("out", [N, M], FP32, kind="ExternalOutput")

    with tile.TileContext(nc) as tc, ExitStack() as ctx:
        consts = ctx.enter_context(tc.tile_pool(name="consts", bufs=1))
        raw = ctx.enter_context(tc.tile_pool(name="raw", bufs=1))
        gated = ctx.enter_context(tc.tile_pool(name="gated", bufs=1))
        small = ctx.enter_context(tc.tile_pool(name="small", bufs=1))
        scratch = ctx.enter_context(tc.tile_pool(name="scratch", bufs=2))
        yn_pool = ctx.enter_context(tc.tile_pool(name="yn", bufs=4))
        tmat = ctx.enter_context(tc.tile_pool(name="tmat", bufs=1))
        ob_pool = ctx.enter_context(tc.tile_pool(name="ob", bufs=3))
        psum_t = ctx.enter_context(tc.tile_pool(name="psum_t", bufs=2, space="PSUM"))
        psum_mm = ctx.enter_context(tc.tile_pool(name="psum_mm", bufs=6, space="PSUM"))

        ident = consts.tile([P, P], FP32)
        make_identity(nc, ident)
        # Force the sqrt_and_others ACT table set (holds Square/Sqrt/Copy —
        # everything we use) to load during the DMA head instead of on the
        # first real Sqrt mid-kernel (~1.3us, unmodeled by the scheduler).
        warm = consts.tile([P, 1], FP32)
        nc.vector.memset(warm, 1.0)
        nc.scalar.sqrt(warm, warm)

        # Bias PSUM evacuations toward ScalarE (~570ns/tile) over VectorE
        # (~658ns/tile): 3-of-8 on DVE keeps both engines below the DMA floor.
        copy_state = {"i": 0}

        def evac(dst, src, scale=None):
            use_vector = (copy_state["i"] % 8) < 3
            copy_state["i"] += 1
            if scale is None:
                if use_vector:
                    nc.vector.tensor_copy(dst, src)
                else:
                    nc.scalar.copy(dst, src)
            else:
                if use_vector:
                    nc.vector.tensor_scalar_mul(dst, src, scale)
                else:
                    nc.scalar.mul(dst, src, scale)

        def rownorm_inv(sums_ap, name, width):
            """inv = 1/sqrt(max(sums, EPS)) on [128, width]; ACT Sqrt is low
            precision (65536 ULP budget) so refine with one Newton step."""
            v = small.tile([P, width], FP32, tag=f"{name}_v")
            s = small.tile([P, width], FP32, tag=f"{name}_s")
            r = small.tile([P, width], FP32, tag=f"{name}_r")
            t = small.tile([P, width], FP32, tag=f"{name}_t")
            inv = small.tile([P, width], FP32, tag=f"{name}_inv")
            nc.vector.tensor_scalar_max(v, sums_ap, EPS)
            nc.scalar.sqrt(s, v)
            nc.vector.reciprocal(r, s)
            nc.vector.tensor_mul(t, v, r)           # t = v/s
            nc.vector.tensor_add(t, t, s)           # t = s + v/s
            nc.vector.tensor_scalar_mul(t, t, 0.5)  # Newton: sqrt(v)
            nc.vector.reciprocal(inv, t)
            return inv

        # ================= loads ============================================
        # X: contiguous permuted layout — row r = 16p + c lives at partition
        # p, sub-tile c; each partition's DMA run is 8KB contiguous.
        # Y: chunk-contiguous — row r = 128c + p, so output columns come out
        # in natural order; loaded in per-group DMAs (group 0 first, since it
        # gates the first column-slice of matmuls).
        Xv = Xd.rearrange("(p c) d -> p c d", p=P)
        A1v = A1d.rearrange("(p c) d -> p c d", p=P)
        # Y block-permuted: row r = 512g + 4p + k -> [p, g, k, :]. Each
        # group-load is 2KB contiguous per partition; group g still covers
        # exactly output column-slice g, and the in-group permutation is
        # undone by stride-4 writes into YnT at evacuation time.
        Yv = Yd.rearrange("(g p k) d -> p g k d", g=NMM, p=P)
        A2v = A2d.rearrange("(g p k) d -> p g k d", g=NMM, p=P)
        xraw = raw.tile([P, NCH, D], FP32, tag="x_raw")
        a1raw = raw.tile([P, NCH, D], FP32, tag="x_araw")
        yraw = raw.tile([P, NMM, NG, D], FP32, tag="y_raw")
        a2raw = raw.tile([P, NMM, NG, D], FP32, tag="y_araw")
        nc.sync.dma_start(out=yraw[:, 0, :, :], in_=Yv[:, 0, :, :])
        nc.sync.dma_start(out=a2raw[:, 0, :, :], in_=A2v[:, 0, :, :])
        for q in range(4):
            sl = slice(q * NG, (q + 1) * NG)
            nc.sync.dma_start(out=xraw[:, sl, :], in_=Xv[:, sl, :])
            nc.sync.dma_start(out=a1raw[:, sl, :], in_=A1v[:, sl, :])
        for g in range(1, NMM):
            nc.sync.dma_start(out=yraw[:, g, :, :], in_=Yv[:, g, :, :])
            nc.sync.dma_start(out=a2raw[:, g, :, :], in_=A2v[:, g, :, :])

        yg = gated.tile([P, NMM, NG, D], FP32, tag="y_g")
        # YnT viewed for the stride-4 un-permuting evacuation writes:
        # column 512g + 4p + k -> [z, g, p, k].

        ysums = small.tile([P, NCH], FP32, tag="y_sums")
        YnT = tmat.tile([P, M], FP32R, tag="YnT")

        def sumsq(g_ap, sums_col, c):
            """Row sum-of-squares of one [128,128] chunk. Alternate engines
            so the norm path doesn't serialize on ACT: even chunks use ACT
            Square w/ accumulator; odd chunks square on GpSimd and reduce on
            DVE. (tensor_tensor_reduce would fuse this but crashes TRN2 HW.)"""
            sq = scratch.tile([P, D], FP32, tag="sq")
            if c % 2 == 0:
                nc.scalar.activation(sq, g_ap, AF.Square, accum_out=sums_col)
            else:
                nc.gpsimd.tensor_mul(sq, g_ap, g_ap)
                nc.vector.reduce_sum(sums_col, sq, axis=mybir.AxisListType.X)

        def make_diag(inv_col, name):
            """diag tile with inv_col ([128,1] AP) on the diagonal; used as
            the matmul moving operand so transpose+row-scale fuse on PE."""
            dg = yn_pool.tile([P, P], FP32, tag="diag")
            bcast = bass.AP(
                tensor=inv_col.tensor,
                offset=inv_col.offset,
                ap=[inv_col.ap[0], [0, P]],
            )
            nc.gpsimd.affine_select(
                out=dg,
                in_=bcast,
                compare_op=mybir.AluOpType.is_equal,
                fill=0.0,
                base=0,
                pattern=[[-1, P]],
                channel_multiplier=1,
            )
            return dg

        YnTv = YnT.rearrange("z (g p k) -> z g p k", g=NMM, k=NG)

        def y_group(g):
            for k in range(NG):
                c = g * NG + k
                eng = nc.vector if c % 2 == 0 else nc.gpsimd
                eng.tensor_mul(yg[:, g, k, :], yraw[:, g, k, :], a2raw[:, g, k, :])
                sumsq(yg[:, g, k, :], ysums[:, c : c + 1], c)
            yinv = rownorm_inv(ysums[:, g * NG : (g + 1) * NG], f"y{g}", NG)
            for k in range(NG):
                yn = yn_pool.tile([P, D], FP32, tag="yn")
                nc.vector.tensor_scalar_mul(yn, yg[:, g, k, :], yinv[:, k : k + 1])
                pt = psum_t.tile([P, P], FP32, tag="pt")
                nc.tensor.transpose(pt, yn, ident)
                evac(YnTv[:, g, :, k], pt)

        # Y group 0 first: its (long) norm chain gates the first
        # column-slice of stage 2.
        y_group(0)

        # ================= X: gate + square + transpose =====================
        # No normalize: 1/norm is folded into stage-2 evacuation. Processed
        # per quarter (4 chunks) with a per-quarter xinv chain, so output
        # row-group n4 of column-slice 0 only waits for X quarter n4.
        xg = gated.tile([P, NCH, D], FP32, tag="x_g")
        XgT = tmat.tile([P, N], FP32R, tag="XgT")
        xsums = small.tile([P, NCH], FP32, tag="x_sums")
        xinv_q = [None] * 4

        def x_quarter(q):
            for k in range(NG):
                c = q * NG + k
                eng = nc.vector if c % 2 == 0 else nc.gpsimd
                eng.tensor_mul(xg[:, c, :], xraw[:, c, :], a1raw[:, c, :])
                sumsq(xg[:, c, :], xsums[:, c : c + 1], c)
                pt = psum_t.tile([P, P], FP32, tag="pt")
                nc.tensor.transpose(pt, xg[:, c, :], ident)
                evac(XgT[:, c * P : (c + 1) * P], pt)
            xinv_q[q] = rownorm_inv(xsums[:, q * NG : (q + 1) * NG], f"x{q}", NG)

        # ================= Y groups interleaved with stage-2 slices =========
        # Column-slice m depends exactly on Y group m; emitting them
        # adjacently keeps engine priority queues aligned with the real
        # dependency order (stage-2 evacs don't wait behind later Y groups).
        OUTv = OUT.rearrange("(p s) m -> p s m", s=SROW)

        def stage2_group(m, n4):
            rhs = YnT[:, m * MM_N : (m + 1) * MM_N]
            ob = ob_pool.tile([P, 4, MM_N], FP32, tag="ob")
            for j in range(4):
                n = n4 * 4 + j
                pm = psum_mm.tile([P, MM_N], FP32, tag="pm")
                nc.tensor.matmul(
                    pm,
                    lhsT=XgT[:, n * P : (n + 1) * P],
                    rhs=rhs,
                    start=True,
                    stop=True,
                )
                if (n4 + j) % 2 == 0:
                    nc.vector.tensor_scalar_mul(
                        ob[:, j, :], pm, xinv_q[n4][:, j : j + 1]
                    )
                else:
                    nc.scalar.mul(ob[:, j, :], pm, xinv_q[n4][:, j : j + 1])
            nc.sync.dma_start(
                out=OUTv[:, n4 * 4 : n4 * 4 + 4, m * MM_N : (m + 1) * MM_N],
                in_=ob,
            )

        # Column-slice 0 interleaved with the X quarters that feed it; each
        # later Y group is emitted before the preceding slice's store stream
        # so its (long) norm chain hides in that slice's DMA window.
        for q in range(4):
            x_quarter(q)
            stage2_group(0, q)
        for m in range(1, NMM):
            y_group(m)
            for n4 in range(4):
                stage2_group(m, n4)

    nc.compile()
    return nc


def _get_program():
    global _CACHED_NC
    if _CACHED_NC is None:
        _CACHED_NC = _build_program()
    return _CACHED_NC


def kernel(X, Y, A_1, A_2, _trace=False, _trace_kwargs=None):
    X = np.asarray(X, dtype=np.float32)
    Y = np.asarray(Y, dtype=np.float32)
    A_1 = np.asarray(A_1, dtype=np.float32)
    A_2 = np.asarray(A_2, dtype=np.float32)
    assert X.shape == (B, N, D), X.shape

    nc = _get_program()
    in_maps = [
        {
            "X": np.ascontiguousarray(X[b]),
            "Y": np.ascontiguousarray(Y[b]),
            "A_1": np.ascontiguousarray(A_1[b]),
            "A_2": np.ascontiguousarray(A_2[b]),
        }
        for b in range(B)
    ]
    res = run_bass_kernel_spmd(
        nc,
        in_maps,
        list(range(B)),
        trace=_trace,
        **(_trace_kwargs or {}),
    )
    out = np.stack([res.results[b]["out"] for b in range(B)], axis=0)
    if _trace:
        return out, res
    return out


# revision 14
# speedup vs baseline: 1.0600x; 1.0600x over previous
"""Gated cosine-affinity kernel for Trainium2 (Bass/Tile), 8-core SPMD.

Problem: for each batch b (B=8):
    Xg = A_1 * X;  Yg = A_2 * Y            (elementwise gates)
    out[b] = normalize_rows(Xg) @ normalize_rows(Yg).T      (2048 x 2048)
with row norm = sqrt(max(|row|^2, 1e-6)).

Sharding: data-parallel over batch — one batch element per NeuronCore.

Per-core structure (memory-bound: ~21 MB HBM traffic vs ~360 GB/s/core):
  stage 1: gate X/Y (DVE+GpSimd), row sum-squares (ACT Square+accum),
           Newton-refined 1/sqrt, PE-transpose into d-major layout.
           X uses a row-permuted contiguous layout (partition p holds rows
           16p..16p+15) so its loads are fully contiguous; the permutation
           is undone for free by a strided store access pattern.
  stage 2: column-slice-major (m-major) matmul order so stores start as
           soon as the first 4 Y chunks are transposed; X's 1/norm is
           folded into the PSUM->SBUF evacuation as a per-partition scale.
           Operands are float32r (1 row/cycle vs 4 for fp32).
"""

import numpy as np
from contextlib import ExitStack

import concourse.bass as bass
import concourse.tile as tile
from concourse import bacc, mybir
from concourse.bass_utils import run_bass_kernel_spmd
from concourse.masks import make_identity

B = 8
N = 2048          # rows of X (output rows)
M = 2048          # rows of Y (output cols)
D = 128           # feature dim == partition count == contraction dim
P = 128
EPS = 1e-6
NCH = N // P      # 16 row-chunks per tensor
NG = 4            # Y chunks per norm-group / per output column-slice
MM_N = 512        # matmul moving free dim (one PSUM bank of fp32)
NMM = M // MM_N   # 4 column-slices
SROW = NCH        # row-permutation stride for X layout

FP32 = mybir.dt.float32
FP32R = mybir.dt.float32r
AF = mybir.ActivationFunctionType

_CACHED_NC = None


def _build_program():
    nc = bacc.Bacc("TRN2", target_bir_lowering=False, debug=False, num_devices=B)

    Xd = nc.dram_tensor("X", [N, D], FP32, kind="ExternalInput")
    Yd = nc.dram_tensor("Y", [M, D], FP32, kind="ExternalInput")
    A1d = nc.dram_tensor("A_1", [N, D], FP32, kind="ExternalInput")
    A2d = nc.dram_tensor("A_2", [M, D], FP32, kind="ExternalInput")
    OUT = nc.dram_tensor("out", [N, M], FP32, kind="ExternalOutput")

    with tile.TileContext(nc) as tc, ExitStack() as ctx:
        consts = ctx.enter_context(tc.tile_pool(name="consts", bufs=1))
        raw = ctx.enter_context(tc.tile_pool(name="raw", bufs=1))
        gated = ctx.enter_context(tc.tile_pool(name="gated", bufs=1))
        small = ctx.enter_context(tc.tile_pool(name="small", bufs=1))
        scratch = ctx.enter_context(tc.tile_pool(name="scratch", bufs=2))
        yn_pool = ctx.enter_context(tc.tile_pool(name="yn", bufs=4))
        tmat = ctx.enter_context(tc.tile_pool(name="tmat", bufs=1))
        ob_pool = ctx.enter_context(tc.tile_pool(name="ob", bufs=3))
        psum_t = ctx.enter_context(tc.tile_pool(name="psum_t", bufs=2, space="PSUM"))
        psum_mm = ctx.enter_context(tc.tile_pool(name="psum_mm", bufs=6, space="PSUM"))

        ident = consts.tile([P, P], FP32)
        make_identity(nc, ident)
        # Force the sqrt_and_others ACT table set (holds Square/Sqrt/Copy —
        # everything we use) to load during the DMA head instead of on the
        # first real Sqrt mid-kernel (~1.3us, unmodeled by the scheduler).
        warm = consts.tile([P, 1], FP32)
        nc.vector.memset(warm, 1.0)
        nc.scalar.sqrt(warm, warm)

        # Bias PSUM evacuations toward ScalarE (~570ns/tile) over VectorE
        # (~658ns/tile): 3-of-8 on DVE keeps both engines below the DMA floor.
        copy_state = {"i": 0}

        def evac(dst, src, scale=None):
            use_vector = (copy_state["i"] % 8) < 3
            copy_state["i"] += 1
            if scale is None:
                if use_vector:
                    nc.vector.tensor_copy(dst, src)
                else:
                    nc.scalar.copy(dst, src)
            else:
                if use_vector:
                    nc.vector.tensor_scalar_mul(dst, src, scale)
                else:
                    nc.scalar.mul(dst, src, scale)

        def rownorm_inv(sums_ap, name, width):
            """inv = 1/sqrt(max(sums, EPS)) on [128, width]; ACT Sqrt is low
            precision (65536 ULP budget) so refine with one Newton step."""
            v = small.tile([P, width], FP32, tag=f"{name}_v")
            s = small.tile([P, width], FP32, tag=f"{name}_s")
            r = small.tile([P, width], FP32, tag=f"{name}_r")
            t = small.tile([P, width], FP32, tag=f"{name}_t")
            inv = small.tile([P, width], FP32, tag=f"{name}_inv")
            nc.vector.tensor_scalar_max(v, sums_ap, EPS)
            nc.scalar.sqrt(s, v)
            nc.vector.reciprocal(r, s)
            nc.vector.tensor_mul(t, v, r)           # t = v/s
            nc.vector.tensor_add(t, t, s)           # t = s + v/s
            nc.vector.tensor_scalar_mul(t, t, 0.5)  # Newton: sqrt(v)
            nc.vector.reciprocal(inv, t)
            return inv

        # ================= loads ============================================
        # X: contiguous permuted layout — row r = 16p + c lives at partition
        # p, sub-tile c; each partition's DMA run is 8KB contiguous.
        # Y: chunk-contiguous — row r = 128c + p, so output columns come out
        # in natural order; loaded in per-group DMAs (group 0 first, since it
        # gates the first column-slice of matmuls).
        Xv = Xd.rearrange("(p c) d -> p c d", p=P)
        A1v = A1d.rearrange("(p c) d -> p c d", p=P)
        # Y block-permuted: row r = 512g + 4p + k -> [p, g, k, :]. Each
        # group-load is 2KB contiguous per partition; group g still covers
        # exactly output column-slice g, and the in-group permutation is
        # undone by stride-4 writes into YnT at evacuation time.
        Yv = Yd.rearrange("(g p k) d -> p g k d", g=NMM, p=P)
        A2v = A2d.rearrange("(g p k) d -> p g k d", g=NMM, p=P)
        xraw = raw.tile([P, NCH, D], FP32, tag="x_raw")
        a1raw = raw.tile([P, NCH, D], FP32, tag="x_araw")
        yraw = raw.tile([P, NMM, NG, D], FP32, tag="y_raw")
        a2raw = raw.tile([P, NMM, NG, D], FP32, tag="y_araw")
        nc.sync.dma_start(out=yraw[:, 0, :, :], in_=Yv[:, 0, :, :])
        nc.sync.dma_start(out=a2raw[:, 0, :, :], in_=A2v[:, 0, :, :])
        for q in range(4):
            sl = slice(q * NG, (q + 1) * NG)
            nc.sync.dma_start(out=xraw[:, sl, :], in_=Xv[:, sl, :])
            nc.sync.dma_start(out=a1raw[:, sl, :], in_=A1v[:, sl, :])
        for g in range(1, NMM):
            nc.sync.dma_start(out=yraw[:, g, :, :], in_=Yv[:, g, :, :])
            nc.sync.dma_start(out=a2raw[:, g, :, :], in_=A2v[:, g, :, :])

        yg = gated.tile([P, NMM, NG, D], FP32, tag="y_g")
        # YnT viewed for the stride-4 un-permuting evacuation writes:
        # column 512g + 4p + k -> [z, g, p, k].

        ysums = small.tile([P, NCH], FP32, tag="y_sums")
        YnT = tmat.tile([P, M], FP32R, tag="YnT")

        def sumsq(g_ap, sums_col, c):
            """Row sum-of-squares of one [128,128] chunk. Alternate engines
            so the norm path doesn't serialize on ACT: even chunks use ACT
            Square w/ accumulator; odd chunks square on GpSimd and reduce on
            DVE. (tensor_tensor_reduce would fuse this but crashes TRN2 HW.)"""
            sq = scratch.tile([P, D], FP32, tag="sq")
            if c % 2 == 0:
                nc.scalar.activation(sq, g_ap, AF.Square, accum_out=sums_col)
            else:
                nc.gpsimd.tensor_mul(sq, g_ap, g_ap)
                nc.vector.reduce_sum(sums_col, sq, axis=mybir.AxisListType.X)

        def make_diag(inv_col, name):
            """diag tile with inv_col ([128,1] AP) on the diagonal; used as
            the matmul moving operand so transpose+row-scale fuse on PE."""
            dg = yn_pool.tile([P, P], FP32, tag="diag")
            bcast = bass.AP(
                tensor=inv_col.tensor,
                offset=inv_col.offset,
                ap=[inv_col.ap[0], [0, P]],
            )
            nc.gpsimd.affine_select(
                out=dg,
                in_=bcast,
                compare_op=mybir.AluOpType.is_equal,
                fill=0.0,
                base=0,
                pattern=[[-1, P]],
                channel_multiplier=1,
            )
            return dg

        YnTv = YnT.rearrange("z (g p k) -> z g p k", g=NMM, k=NG)

        def y_group(g):
            for k in range(NG):
                c = g * NG + k
                nc.gpsimd.tensor_mul(yg[:, g, k, :], yraw[:, g, k, :], a2raw[:, g, k, :])
                sumsq(yg[:, g, k, :], ysums[:, c : c + 1], c)
            yinv = rownorm_inv(ysums[:, g * NG : (g + 1) * NG], f"y{g}", NG)
            for k in range(NG):
                yn = yn_pool.tile([P, D], FP32, tag="yn")
                nc.vector.tensor_scalar_mul(yn, yg[:, g, k, :], yinv[:, k : k + 1])
                pt = psum_t.tile([P, P], FP32, tag="pt")
                nc.tensor.transpose(pt, yn, ident)
                evac(YnTv[:, g, :, k], pt)

        # Y group 0 first: its (long) norm chain gates the first
        # column-slice of stage 2.
        y_group(0)

        # ================= X: gate + square + transpose =====================
        # No normalize: 1/norm is folded into stage-2 evacuation. Processed
        # per quarter (4 chunks) with a per-quarter xinv chain, so output
        # row-group n4 of column-slice 0 only waits for X quarter n4.
        xg = gated.tile([P, NCH, D], FP32, tag="x_g")
        XgT = tmat.tile([P, N], FP32R, tag="XgT")
        xsums = small.tile([P, NCH], FP32, tag="x_sums")
        xinv_q = [None] * 4

        def x_quarter(q):
            for k in range(NG):
                c = q * NG + k
                nc.gpsimd.tensor_mul(xg[:, c, :], xraw[:, c, :], a1raw[:, c, :])
                sumsq(xg[:, c, :], xsums[:, c : c + 1], c)
                pt = psum_t.tile([P, P], FP32, tag="pt")
                nc.tensor.transpose(pt, xg[:, c, :], ident)
                evac(XgT[:, c * P : (c + 1) * P], pt)
            xinv_q[q] = rownorm_inv(xsums[:, q * NG : (q + 1) * NG], f"x{q}", NG)

        # ================= Y groups interleaved with stage-2 slices =========
        # Column-slice m depends exactly on Y group m; emitting them
        # adjacently keeps engine priority queues aligned with the real
        # dependency order (stage-2 evacs don't wait behind later Y groups).
        OUTv = OUT.rearrange("(p s) m -> p s m", s=SROW)

        def stage2_group(m, n4):
            rhs = YnT[:, m * MM_N : (m + 1) * MM_N]
            ob = ob_pool.tile([P, 4, MM_N], FP32, tag="ob")
            for j in range(4):
                n = n4 * 4 + j
                pm = psum_mm.tile([P, MM_N], FP32, tag="pm")
                nc.tensor.matmul(
                    pm,
                    lhsT=XgT[:, n * P : (n + 1) * P],
                    rhs=rhs,
                    start=True,
                    stop=True,
                )
                if (n4 + j) % 2 == 0:
                    nc.vector.tensor_scalar_mul(
                        ob[:, j, :], pm, xinv_q[n4][:, j : j + 1]
                    )
                else:
                    nc.scalar.mul(ob[:, j, :], pm, xinv_q[n4][:, j : j + 1])
            nc.sync.dma_start(
                out=OUTv[:, n4 * 4 : n4 * 4 + 4, m * MM_N : (m + 1) * MM_N],
                in_=ob,
            )

        # Column-slice 0 interleaved with the X quarters that feed it; each
        # later Y group is emitted before the preceding slice's store stream
        # so its (long) norm chain hides in that slice's DMA window.
        for q in range(4):
            x_quarter(q)
            stage2_group(0, q)
        for m in range(1, NMM):
            y_group(m)
            for n4 in range(4):
                stage2_group(m, n4)

    nc.compile()
    return nc


def _get_program():
    global _CACHED_NC
    if _CACHED_NC is None:
        _CACHED_NC = _build_program()
    return _CACHED_NC


def kernel(X, Y, A_1, A_2, _trace=False, _trace_kwargs=None):
    X = np.asarray(X, dtype=np.float32)
    Y = np.asarray(Y, dtype=np.float32)
    A_1 = np.asarray(A_1, dtype=np.float32)
    A_2 = np.asarray(A_2, dtype=np.float32)
    assert X.shape == (B, N, D), X.shape

    nc = _get_program()
    in_maps = [
        {
            "X": np.ascontiguousarray(X[b]),
            "Y": np.ascontiguousarray(Y[b]),
            "A_1": np.ascontiguousarray(A_1[b]),
            "A_2": np.ascontiguousarray(A_2[b]),
        }
        for b in range(B)
    ]
    res = run_bass_kernel_spmd(
        nc,
        in_maps,
        list(range(B)),
        trace=_trace,
        **(_trace_kwargs or {}),
    )
    out = np.stack([res.results[b]["out"] for b in range(B)], axis=0)
    if _trace:
        return out, res
    return out
